# revision 5
# baseline (speedup 1.0000x reference)
"""Trainium2 Bass kernel for CRFDecoder.fit (sum reduction).

Math (scaled forward algorithm, probability space, no padding mask):
  q_0[j,b]  = exp(start[j] + em[0,b,j])                     (bf16, tag-major)
  q_t       = (expT.T @ q_{t-1}) * exp(em_t - LAM)          (PE bf16 matmuls, fp32 PSUM)
  z_t[b]    = sum_j q_t[j,b] * expEnd[j]
  logZ_b    = ln(z_{len_b-1}[b]) + LAM*(len_b-1)
  score_b   = host-indexed tables reduced on device
  out       = sum_b (logZ_b - score_b)                      (per-core partial; host sums 8)

v2 structure (vs v1):
  - 2 antiphase batch groups of 8 columns each; group g1 runs half a step
    behind g0 so its PE matmul block overlaps g0's DVE multiply + sync
    latency (and vice versa). The per-step z matmuls are gone.
  - Every step's state q_t is written into a persistent history buffer
    qall_g[j_lo, h, b, t] (bf16, 2 MB/group) by the DVE multiply itself.
  - z[b, t] for all t is computed from qall by chunked M=1 matmuls
    (lhsT = expEnd) interleaved into the scan's PE idle slots, assembled
    into a [16, 512] tile via small PSUM->SBUF DMAs, then the one-hot
    select/ln/score tail runs as in v1.

Sharding: data-parallel over batch: core c handles batch columns [16c, 16c+16).
Tag dim 256 is split as j = h*128 + j_lo (h in {0,1}).
Emission is host-rearranged per core to [j_lo, t, h, b] bf16 so every DMA is
contiguous.
"""

import os

import numpy as np
import ml_dtypes

SLN, BSZ, TAG = 512, 128, 256
NCORES = 8
B = BSZ // NCORES          # 16 per-core batch
P = 128                    # partitions
H = TAG // P               # 2 tag halves
G = 2                      # pipeline groups
BG = B // G                # 8 columns per group
LAM = float(np.log(TAG) + 0.5)
EM_N = P * SLN * H * B     # flattened emission elements per core

bf16 = ml_dtypes.bfloat16

_CACHE: dict = {}


def _build_bass():
    import concourse.bacc as bacc
    import concourse.tile as tile
    from concourse import mybir
    import concourse.bass as bass

    nc = bacc.Bacc(
        "TRN2",
        target_bir_lowering=False,
        debug=False,
        enable_asserts=False,
        num_devices=NCORES,
    )
    f32 = mybir.dt.float32
    bft = mybir.dt.bfloat16

    em_h = nc.dram_tensor("em", [EM_N], bft, kind="ExternalInput")
    expT_h = nc.dram_tensor("expT", [H, H, P, P], bft, kind="ExternalInput")
    startT_h = nc.dram_tensor("startT", [P, H], f32, kind="ExternalInput")
    expEnd_h = nc.dram_tensor("expEnd", [P, H], bft, kind="ExternalInput")
    lamlen_h = nc.dram_tensor("lamlen", [B, 1], f32, kind="ExternalInput")
    onehot_h = nc.dram_tensor("onehot", [B, SLN], f32, kind="ExternalInput")
    emv_h = nc.dram_tensor("emv", [B, SLN], f32, kind="ExternalInput")
    tv_h = nc.dram_tensor("tv", [B, SLN + 1], f32, kind="ExternalInput")
    emm_h = nc.dram_tensor("emm", [B, SLN], f32, kind="ExternalInput")
    tm_h = nc.dram_tensor("tm", [B, SLN + 1], f32, kind="ExternalInput")
    out_h = nc.dram_tensor("out", [1, 1], f32, kind="ExternalOutput")

    em_view = em_h.ap()[:EM_N].rearrange(
        "(p t h b) -> p t h b", p=P, t=SLN, h=H, b=B
    )

    NSB = 8                 # emission superblocks
    SBL = SLN // NSB        # 64 steps per superblock
    ZCH = 8                 # z chunks per group (64 steps each)
    ZCL = SLN // ZCH

    from contextlib import ExitStack

    with tile.TileContext(nc) as tc, ExitStack() as es:
        persist = es.enter_context(tc.tile_pool(name="persist", bufs=1))

        def st(shape, dtype, name):
            return persist.tile(shape, dtype, name=name, tag=name)

        # ---- constants ----
        expT_sb = st([P, H, H, P], bft, name="expT_sb")   # (i_lo, k, h, j_lo)
        for k in range(H):
            for h in range(H):
                nc.sync.dma_start(out=expT_sb[:, k, h, :], in_=expT_h.ap()[k, h, :, :])
        startT_sb = st([P, H], f32, name="startT_sb")
        nc.sync.dma_start(out=startT_sb, in_=startT_h.ap())
        expEnd_sb = st([P, H], bft, name="expEnd_sb")
        nc.sync.dma_start(out=expEnd_sb, in_=expEnd_h.ap())
        lamlen_sb = st([B, 1], f32, name="lamlen_sb")
        nc.sync.dma_start(out=lamlen_sb, in_=lamlen_h.ap())
        onehot_sb = st([B, SLN], f32, name="onehot_sb")
        nc.sync.dma_start(out=onehot_sb, in_=onehot_h.ap())
        emv_sb = st([B, SLN], f32, name="emv_sb")
        nc.sync.dma_start(out=emv_sb, in_=emv_h.ap())
        tv_sb = st([B, SLN + 1], f32, name="tv_sb")
        nc.sync.dma_start(out=tv_sb, in_=tv_h.ap())
        emm_sb = st([B, SLN], f32, name="emm_sb")
        nc.sync.dma_start(out=emm_sb, in_=emm_h.ap())
        tm_sb = st([B, SLN + 1], f32, name="tm_sb")
        nc.sync.dma_start(out=tm_sb, in_=tm_h.ap())
        ones_sb = st([B, 1], f32, name="ones_sb")
        nc.vector.memset(ones_sb, 1.0)
        neglam_sb = st([P, 1], f32, name="neglam_sb")
        nc.vector.memset(neglam_sb, -LAM)

        # ---- emission load + exp (prefetched per superblock) ----
        em_t = []
        expem_t = []
        emp = es.enter_context(tc.tile_pool(name="emp", bufs=NSB))
        exq = es.enter_context(tc.tile_pool(name="exp", bufs=NSB))
        for i in range(NSB):
            emt = emp.tile([P, SBL, H, B], bft, tag="emt")
            nc.sync.dma_start(
                out=emt, in_=em_view[:, i * SBL : (i + 1) * SBL, :, :]
            )
            em_t.append(emt)
            xt = exq.tile([P, SBL, H, B], bft, tag="xt")
            nc.scalar.activation(
                xt,
                emt,
                mybir.ActivationFunctionType.Exp,
                bias=neglam_sb[:],
                scale=1.0,
            )
            expem_t.append(xt)

        # ---- per-group state history ----
        # qall_g layout: [j_lo, h, b, t] so the z-chunk matmuls get a
        # contiguous t run per (h, b).
        qall = [st([P, H, BG, SLN], bft, name=f"qall{g}") for g in range(G)]

        up = [
            es.enter_context(tc.tile_pool(name=f"up{g}", bufs=2, space="PSUM"))
            for g in range(G)
        ]
        zp = es.enter_context(tc.tile_pool(name="zp", bufs=2, space="PSUM"))
        zbuf = st([B, SLN], f32, name="zbuf")
        # partition-0 staging row for z chunks: [1, g, b, ci, t] so one flat
        # SBUF->SBUF DMA scatters it to zbuf's [b16, t] layout
        zrow = st([1, G, BG, ZCH, ZCL], f32, name="zrow")

        # q0 = exp(em0 + start)
        for g in range(G):
            for h in range(H):
                nc.scalar.activation(
                    qall[g][:, h, :, 0],
                    em_t[0][:, 0, h, g * BG : (g + 1) * BG],
                    mybir.ActivationFunctionType.Exp,
                    bias=startT_sb[:, h : h + 1],
                    scale=1.0,
                )

        def z_chunk(g, ci):
            """z[b, t] for group g, t in [ci*ZCL, (ci+1)*ZCL): M=1 matmuls
            accumulating over h, then a Scalar-engine PSUM->SBUF copy into
            the staging row (DMA cannot read PSUM)."""
            zc = zp.tile([1, BG, ZCL], mybir.dt.float32, tag="zc")
            for h in range(H):
                nc.tensor.matmul(
                    zc[:, :, :],
                    expEnd_sb[:, h : h + 1],
                    qall[g][:, h, :, ci * ZCL : (ci + 1) * ZCL],
                    start=(h == 0),
                    stop=(h == H - 1),
                )
            nc.scalar.copy(zrow[:, g, :, ci, :], zc)

        NSTEPS = int(os.environ.get("CRF_STEPS", SLN))
        for t in range(1, NSTEPS):
            sb, col = divmod(t, SBL)
            for g in range(G):
                u = up[g].tile([P, H, BG], mybir.dt.float32, tag="u")
                for h in range(H):
                    for k in range(H):
                        nc.tensor.matmul(
                            u[:, h, :],
                            expT_sb[:, k, h, :],
                            qall[g][:, k, :, t - 1],
                            start=(k == 0),
                            stop=(k == H - 1),
                        )
                nc.vector.tensor_mul(
                    qall[g][:, :, :, t],
                    u,
                    expem_t[sb][:, col, :, g * BG : (g + 1) * BG],
                )
            # interleave finished z chunks into the scan's PE stream
            if t % ZCL == 0 and t >= ZCL:
                ci = t // ZCL - 1
                for g in range(G):
                    z_chunk(g, ci)

        # final z chunk (and any not yet emitted)
        done = (NSTEPS - 1) // ZCL
        for ci in range(done, ZCH):
            for g in range(G):
                z_chunk(g, ci)

        # scatter the staging row into [b16, t] layout (cross-partition)
        nc.sync.dma_start(out=zbuf, in_=zrow)

        # ---- finalization ----
        prod_sb = st([B, SLN], mybir.dt.float32, name="prod_sb")
        z_sel = st([B, 1], mybir.dt.float32, name="z_sel")
        nc.vector.tensor_mul(prod_sb, zbuf, onehot_sb)
        nc.vector.reduce_sum(z_sel, prod_sb, axis=mybir.AxisListType.X)
        logz = st([B, 1], mybir.dt.float32, name="logz")
        nc.scalar.activation(logz, z_sel, mybir.ActivationFunctionType.Ln)
        logz2 = st([B, 1], mybir.dt.float32, name="logz2")
        nc.vector.tensor_add(logz2, logz, lamlen_sb)

        emprod = st([B, SLN], mybir.dt.float32, name="emprod")
        em_part = st([B, 1], mybir.dt.float32, name="em_part")
        nc.vector.tensor_mul(emprod, emv_sb, emm_sb)
        nc.vector.reduce_sum(em_part, emprod, axis=mybir.AxisListType.X)
        tprod = st([B, SLN + 1], mybir.dt.float32, name="tprod")
        t_part = st([B, 1], mybir.dt.float32, name="t_part")
        nc.vector.tensor_mul(tprod, tv_sb, tm_sb)
        nc.vector.reduce_sum(t_part, tprod, axis=mybir.AxisListType.X)

        score = st([B, 1], mybir.dt.float32, name="score")
        nc.vector.tensor_add(score, em_part, t_part)
        res = st([B, 1], mybir.dt.float32, name="res")
        nc.vector.tensor_sub(res, logz2, score)

        tp = es.enter_context(tc.tile_pool(name="tp", bufs=1, space="PSUM"))
        tot_ps = tp.tile([1, 1], mybir.dt.float32)
        nc.tensor.matmul(tot_ps, res, ones_sb, start=True, stop=True)
        tot_sb = st([1, 1], mybir.dt.float32, name="tot_sb")
        nc.vector.tensor_copy(tot_sb, tot_ps)
        nc.sync.dma_start(out=out_h.ap(), in_=tot_sb)

    nc.compile()
    return nc


def _prep_inputs(emission, length, target, transition, start_transition, end_transition):
    """Host-side sharding/layout prep. Returns list of per-core input dicts."""
    emission = np.asarray(emission, np.float32)
    length = np.asarray(length).astype(np.int64)
    target = np.asarray(target).astype(np.int64)
    T = np.asarray(transition, np.float32)
    startT = np.asarray(start_transition, np.float32)
    endT = np.asarray(end_transition, np.float32)

    expT_full = np.exp(T, dtype=np.float32)
    expT_arr = np.zeros((H, H, P, P), bf16)
    for k in range(H):
        for h in range(H):
            expT_arr[k, h] = expT_full[k * P : (k + 1) * P, h * P : (h + 1) * P].astype(
                bf16
            )
    startT_arr = np.ascontiguousarray(
        startT.reshape(H, P).T, dtype=np.float32
    )  # [j_lo, h]
    expEnd_arr = np.ascontiguousarray(np.exp(endT).reshape(H, P).T).astype(bf16)

    in_maps = []
    for c in range(NCORES):
        bs = slice(c * B, (c + 1) * B)
        emc = emission[:, bs, :]                    # [512,16,256]
        lenc = length[bs]                           # [16]
        tgt = target[:, bs]                         # [512,16]

        # [j_lo, t, h, b] layout, contiguous (h,b) runs of 64B
        em_r = np.transpose(
            emc.reshape(SLN, B, H, P), (3, 0, 2, 1)
        )  # [j_lo, t, h, b]
        em_arr = np.ascontiguousarray(em_r).astype(bf16).ravel()

        tt = np.arange(SLN)[:, None]
        pad = tt >= lenc[None, :]                   # [512,16]
        bb = np.arange(B)

        # score tables: host does PURE INDEXING; all arithmetic on device
        emv = np.take_along_axis(emc, tgt[:, :, None], axis=2)[:, :, 0].T  # [16,512]
        emv = np.ascontiguousarray(emv, np.float32)
        emm = np.ascontiguousarray((~pad).T, np.float32)          # [16,512]
        tv = np.zeros((B, SLN + 1), np.float32)
        tv[:, 0] = startT[tgt[0]]
        tv[:, 1:SLN] = T[tgt[:-1], tgt[1:]].T
        tv[:, SLN] = endT[tgt[lenc - 1, bb]]
        tm = np.ones((B, SLN + 1), np.float32)
        tm[:, 1:SLN] = (~pad[1:]).T

        onehot = np.zeros((B, SLN), np.float32)
        onehot[bb, lenc - 1] = 1.0
        lamlen = (LAM * (lenc - 1)).astype(np.float32).reshape(B, 1)

        in_maps.append(
            dict(
                em=em_arr,
                expT=expT_arr,
                startT=startT_arr,
                expEnd=expEnd_arr,
                lamlen=lamlen,
                onehot=onehot,
                emv=emv,
                tv=tv,
                emm=emm,
                tm=tm,
            )
        )
    return in_maps


def kernel(
    emission,
    length,
    padding_mask,
    target,
    transition,
    start_transition,
    end_transition,
):
    from concourse import bass_utils

    in_maps = _prep_inputs(
        emission, length, target, transition, start_transition, end_transition
    )
    if "nc" not in _CACHE:
        _CACHE["nc"] = _build_bass()
    nc = _CACHE["nc"]
    res = bass_utils.run_bass_kernel_spmd(
        nc, in_maps, core_ids=list(range(NCORES))
    )
    total = np.float32(0.0)
    for c in range(NCORES):
        total += np.float32(res.results[c]["out"].reshape(-1)[0])
    return np.asarray(total, dtype=np.float32)


# revision 6
# speedup vs baseline: 2.0619x; 2.0619x over previous
"""Trainium2 Bass kernel for CRFDecoder.fit (sum reduction).

v3: meet-in-the-middle scan. The 511-step forward recursion is replaced by
two INDEPENDENT 256-step chains that run concurrently, halving the serial
chain-latency wall (the per-step MM->DVE->MM latency is irreducible, so the
win comes from needing half as many sequential steps):

  F chain (forward):      qF_t = (T' qF_{t-1}) * eF_t      t = 0..SF
  R chain (time-reversed): qR_u = (T  qR_{u-1}) * eR_u      u = 0..SR

Both start from the all-ones state. Host crafts per-column emission streams:
  - dummy steps  e = 1/colsum  hold the state exactly at ones (the ones
    vector is the dominant eigendirection of the near-ones expT, so this
    fixed point is numerically stable, unlike any expEnd-based one),
  - a seed step  e = exp(start + em_0)/colsum  injects the true alpha_0
    (resp. exp(end + em_{L-1}) for the R chain) at a per-column offset,
  - real steps   e = exp(em_t - LAM)  as usual.
Per column: nF + nR = L-2 real transitions split across the chains, dummies
front-pad both streams so EVERY column meets at the fixed step (SF, SR):

  Z_b * e^{-LAM (L_b-2)} = sum_j (T' qF_SF)[j,b] * qR_SR[j,b]

The bridge T' apply is one extra MM block; the meet is one DVE mul plus two
ones-matmuls. No state history, no per-t z readout, no gathers.

Sharding: data-parallel over batch: core c handles batch columns [16c, 16c+16).
Tag dim 256 is split as j = h*128 + j_lo (h in {0,1}).
"""

import os

import numpy as np
import ml_dtypes

SLN, BSZ, TAG = 512, 128, 256
NCORES = 8
B = BSZ // NCORES          # 16 per-core batch
P = 128                    # partitions
H = TAG // P               # 2 tag halves
LAM = float(np.log(TAG) + 0.5)
SF = 255                   # F chain runs steps 0..SF
SR = 255                   # R chain runs steps 0..SR
NCH = SF + 1 + SR + 1      # combined stream length (F then R) = 512
EM_N = P * NCH * H * B     # flattened emission elements per core

bf16 = ml_dtypes.bfloat16

_CACHE: dict = {}


def _build_bass():
    import concourse.bacc as bacc
    import concourse.tile as tile
    from concourse import mybir

    nc = bacc.Bacc(
        "TRN2",
        target_bir_lowering=False,
        debug=False,
        enable_asserts=False,
        num_devices=NCORES,
    )
    f32 = mybir.dt.float32
    bft = mybir.dt.bfloat16

    em_h = nc.dram_tensor("em", [EM_N], bft, kind="ExternalInput")
    expT_h = nc.dram_tensor("expT", [H, H, P, P], bft, kind="ExternalInput")
    expTT_h = nc.dram_tensor("expTT", [H, H, P, P], bft, kind="ExternalInput")
    lamlen_h = nc.dram_tensor("lamlen", [B, 1], f32, kind="ExternalInput")
    emv_h = nc.dram_tensor("emv", [B, SLN], f32, kind="ExternalInput")
    tv_h = nc.dram_tensor("tv", [B, SLN + 1], f32, kind="ExternalInput")
    emm_h = nc.dram_tensor("emm", [B, SLN], f32, kind="ExternalInput")
    tm_h = nc.dram_tensor("tm", [B, SLN + 1], f32, kind="ExternalInput")
    out_h = nc.dram_tensor("out", [1, 1], f32, kind="ExternalOutput")

    em_view = em_h.ap()[:EM_N].rearrange(
        "(p t h b) -> p t h b", p=P, t=NCH, h=H, b=B
    )

    NSB = 8                 # emission superblocks (4 per chain)
    SBL = NCH // NSB        # 64 steps per superblock

    from contextlib import ExitStack

    with tile.TileContext(nc) as tc, ExitStack() as es:
        persist = es.enter_context(tc.tile_pool(name="persist", bufs=1))

        def st(shape, dtype, name):
            return persist.tile(shape, dtype, name=name, tag=name)

        # ---- constants ----
        expT_sb = st([P, H, H, P], bft, name="expT_sb")   # (i_lo, k, h, j_lo)
        expTT_sb = st([P, H, H, P], bft, name="expTT_sb")
        for k in range(H):
            for h in range(H):
                nc.sync.dma_start(out=expT_sb[:, k, h, :], in_=expT_h.ap()[k, h, :, :])
                nc.sync.dma_start(out=expTT_sb[:, k, h, :], in_=expTT_h.ap()[k, h, :, :])
        lamlen_sb = st([B, 1], f32, name="lamlen_sb")
        nc.sync.dma_start(out=lamlen_sb, in_=lamlen_h.ap())
        emv_sb = st([B, SLN], f32, name="emv_sb")
        nc.sync.dma_start(out=emv_sb, in_=emv_h.ap())
        tv_sb = st([B, SLN + 1], f32, name="tv_sb")
        nc.sync.dma_start(out=tv_sb, in_=tv_h.ap())
        emm_sb = st([B, SLN], f32, name="emm_sb")
        nc.sync.dma_start(out=emm_sb, in_=emm_h.ap())
        tm_sb = st([B, SLN + 1], f32, name="tm_sb")
        nc.sync.dma_start(out=tm_sb, in_=tm_h.ap())
        ones_sb = st([B, 1], f32, name="ones_sb")
        nc.vector.memset(ones_sb, 1.0)
        onesP_sb = st([P, 1], bft, name="onesP_sb")
        nc.vector.memset(onesP_sb, 1.0)
        neglam_sb = st([P, 1], f32, name="neglam_sb")
        nc.vector.memset(neglam_sb, -LAM)
        qinit = st([P, H, B], bft, name="qinit")
        nc.vector.memset(qinit, 1.0)

        # ---- emission load + exp (prefetched per superblock) ----
        # superblocks 0..3 = F chain steps, 4..7 = R chain steps; prefetch
        # interleaved so both chains' first blocks arrive first.
        em_t = [None] * NSB
        expem_t = [None] * NSB
        emp = es.enter_context(tc.tile_pool(name="emp", bufs=NSB))
        exq = es.enter_context(tc.tile_pool(name="exp", bufs=NSB))
        for i in (0, 4, 1, 5, 2, 6, 3, 7):
            emt = emp.tile([P, SBL, H, B], bft, tag="emt")
            nc.sync.dma_start(
                out=emt, in_=em_view[:, i * SBL : (i + 1) * SBL, :, :]
            )
            em_t[i] = emt
            xt = exq.tile([P, SBL, H, B], bft, tag="xt")
            nc.scalar.activation(
                xt,
                emt,
                mybir.ActivationFunctionType.Exp,
                bias=neglam_sb[:],
                scale=1.0,
            )
            expem_t[i] = xt

        qfp = es.enter_context(tc.tile_pool(name="qfp", bufs=3))
        qrp = es.enter_context(tc.tile_pool(name="qrp", bufs=3))
        upF = es.enter_context(tc.tile_pool(name="upF", bufs=2, space="PSUM"))
        upR = es.enter_context(tc.tile_pool(name="upR", bufs=2, space="PSUM"))

        def step(qprev, t_sb, wt, up, qp):
            sb, col = t_sb
            u = up.tile([P, H, B], mybir.dt.float32, tag="u")
            for h in range(H):
                for k in range(H):
                    nc.tensor.matmul(
                        u[:, h, :],
                        wt[:, k, h, :],
                        qprev[:, k, :],
                        start=(k == 0),
                        stop=(k == H - 1),
                    )
            qn = qp.tile([P, H, B], bft, tag="q")
            nc.vector.tensor_mul(qn, u, expem_t[sb][:, col, :, :])
            return qn

        NSTEPS = int(os.environ.get("CRF_STEPS", SF + 1))
        qf, qr = qinit, qinit
        for t in range(NSTEPS):
            qf = step(qf, divmod(t, SBL), expT_sb, upF, qfp)
            qr = step(qr, divmod(SF + 1 + t, SBL), expTT_sb, upR, qrp)

        # ---- bridge + meet ----
        uF = upF.tile([P, H, B], mybir.dt.float32, tag="u")
        for h in range(H):
            for k in range(H):
                nc.tensor.matmul(
                    uF[:, h, :],
                    expT_sb[:, k, h, :],
                    qf[:, k, :],
                    start=(k == 0),
                    stop=(k == H - 1),
                )
        meet = st([P, H, B], bft, name="meet")
        nc.vector.tensor_mul(meet, uF, qr)

        zp = es.enter_context(tc.tile_pool(name="zp", bufs=1, space="PSUM"))
        z_ps = zp.tile([1, B], mybir.dt.float32)
        for h in range(H):
            nc.tensor.matmul(
                z_ps,
                onesP_sb,
                meet[:, h, :],
                start=(h == 0),
                stop=(h == H - 1),
            )
        z_row = st([1, B], f32, name="z_row")
        nc.scalar.copy(z_row, z_ps)
        z_sel = st([B, 1], f32, name="z_sel")
        nc.sync.dma_start(out=z_sel, in_=z_row)

        # ---- finalization ----
        logz = st([B, 1], f32, name="logz")
        nc.scalar.activation(logz, z_sel, mybir.ActivationFunctionType.Ln)
        logz2 = st([B, 1], f32, name="logz2")
        nc.vector.tensor_add(logz2, logz, lamlen_sb)

        emprod = st([B, SLN], f32, name="emprod")
        em_part = st([B, 1], f32, name="em_part")
        nc.vector.tensor_mul(emprod, emv_sb, emm_sb)
        nc.vector.reduce_sum(em_part, emprod, axis=mybir.AxisListType.X)
        tprod = st([B, SLN + 1], f32, name="tprod")
        t_part = st([B, 1], f32, name="t_part")
        nc.vector.tensor_mul(tprod, tv_sb, tm_sb)
        nc.vector.reduce_sum(t_part, tprod, axis=mybir.AxisListType.X)

        score = st([B, 1], f32, name="score")
        nc.vector.tensor_add(score, em_part, t_part)
        res = st([B, 1], f32, name="res")
        nc.vector.tensor_sub(res, logz2, score)

        tp = es.enter_context(tc.tile_pool(name="tp", bufs=1, space="PSUM"))
        tot_ps = tp.tile([1, 1], mybir.dt.float32)
        nc.tensor.matmul(tot_ps, res, ones_sb, start=True, stop=True)
        tot_sb = st([1, 1], f32, name="tot_sb")
        nc.vector.tensor_copy(tot_sb, tot_ps)
        nc.sync.dma_start(out=out_h.ap(), in_=tot_sb)

    nc.compile()
    return nc


def _prep_inputs(emission, length, target, transition, start_transition, end_transition):
    """Host-side sharding/layout prep. Returns list of per-core input dicts."""
    emission = np.asarray(emission, np.float32)
    length = np.asarray(length).astype(np.int64)
    target = np.asarray(target).astype(np.int64)
    T = np.asarray(transition, np.float32)
    startT = np.asarray(start_transition, np.float32)
    endT = np.asarray(end_transition, np.float32)

    expT_full = np.exp(T).astype(bf16).astype(np.float32)
    lnc_col = np.log(expT_full.sum(axis=0)).astype(np.float32)  # for T' q
    lnc_row = np.log(expT_full.sum(axis=1)).astype(np.float32)  # for T  r

    def tiles(M):
        arr = np.zeros((H, H, P, P), bf16)
        for k in range(H):
            for h in range(H):
                arr[k, h] = M[k * P : (k + 1) * P, h * P : (h + 1) * P].astype(bf16)
        return arr

    expT_arr = tiles(expT_full)
    expTT_arr = tiles(np.ascontiguousarray(expT_full.T))

    in_maps = []
    for c in range(NCORES):
        bs = slice(c * B, (c + 1) * B)
        emc = emission[:, bs, :]                    # [512,16,256]
        lenc = length[bs]                           # [16]
        tgt = target[:, bs]                         # [512,16]
        bb = np.arange(B)

        # ---- build F and R emission streams [steps, b, tag] ----
        nF = np.minimum(lenc - 2, SF)               # [16]
        nR = lenc - 2 - nF
        dF = SF - nF
        dR = SR - nR

        tauF = np.arange(SF + 1)[:, None]           # [256,16]
        posF = np.clip(tauF - dF[None, :], 0, SLN - 1)  # real position, 0 at seed
        emF = np.take_along_axis(
            emc, posF[:, :, None], axis=0
        )  # [256,16,256] = em[posF[t,b], b, :]
        dummyF = (LAM - lnc_col)[None, None, :]
        mF_dummy = (tauF < dF[None, :])[:, :, None]
        mF_seed = (tauF == dF[None, :])[:, :, None]
        emF = np.where(mF_dummy, dummyF, emF)
        emF = np.where(
            mF_seed, emF + (startT - lnc_col + LAM)[None, None, :], emF
        )

        tauR = np.arange(SR + 1)[:, None]
        posR = np.clip(
            (lenc - 1)[None, :] - (tauR - dR[None, :]), 0, SLN - 1
        )
        posR = np.where(tauR <= dR[None, :], (lenc - 1)[None, :], posR)
        emR = np.take_along_axis(emc, posR[:, :, None], axis=0)
        dummyR = (LAM - lnc_row)[None, None, :]
        mR_dummy = (tauR < dR[None, :])[:, :, None]
        mR_seed = (tauR == dR[None, :])[:, :, None]
        emR = np.where(mR_dummy, dummyR, emR)
        emR = np.where(
            mR_seed, emR + (endT - lnc_row + LAM)[None, None, :], emR
        )

        emFR = np.concatenate([emF, emR], axis=0)   # [512,16,256]
        em_r = np.transpose(
            emFR.reshape(NCH, B, H, P), (3, 0, 2, 1)
        )  # [j_lo, t, h, b]
        em_arr = np.ascontiguousarray(em_r).astype(bf16).ravel()

        # ---- score tables (host does PURE INDEXING; arithmetic on device) ----
        tt = np.arange(SLN)[:, None]
        pad = tt >= lenc[None, :]                   # [512,16]
        emv = np.take_along_axis(emc, tgt[:, :, None], axis=2)[:, :, 0].T
        emv = np.ascontiguousarray(emv, np.float32)
        emm = np.ascontiguousarray((~pad).T, np.float32)
        tv = np.zeros((B, SLN + 1), np.float32)
        tv[:, 0] = startT[tgt[0]]
        tv[:, 1:SLN] = T[tgt[:-1], tgt[1:]].T
        tv[:, SLN] = endT[tgt[lenc - 1, bb]]
        tm = np.ones((B, SLN + 1), np.float32)
        tm[:, 1:SLN] = (~pad[1:]).T

        lamlen = (LAM * (lenc - 2)).astype(np.float32).reshape(B, 1)

        in_maps.append(
            dict(
                em=em_arr,
                expT=expT_arr,
                expTT=expTT_arr,
                lamlen=lamlen,
                emv=emv,
                tv=tv,
                emm=emm,
                tm=tm,
            )
        )
    return in_maps


def kernel(
    emission,
    length,
    padding_mask,
    target,
    transition,
    start_transition,
    end_transition,
):
    from concourse import bass_utils

    in_maps = _prep_inputs(
        emission, length, target, transition, start_transition, end_transition
    )
    if "nc" not in _CACHE:
        _CACHE["nc"] = _build_bass()
    nc = _CACHE["nc"]
    res = bass_utils.run_bass_kernel_spmd(
        nc, in_maps, core_ids=list(range(NCORES))
    )
    total = np.float32(0.0)
    for c in range(NCORES):
        total += np.float32(res.results[c]["out"].reshape(-1)[0])
    return np.asarray(total, dtype=np.float32)
